# revision 38
# baseline (speedup 1.0000x reference)
"""2D Daubechies-2 DWT on Trainium2 — all-TensorE design, bf16, 8-core DP.

Input  x: [16, 1024, 1024, 1] f32  ->  Output: [16, 512, 512, 4] f32
Per core: 2 images. Host casts input to bf16 and builds one tiny banded
filter matrix W [128 x 260] (t=0 mirror variant | interior variant, each
[128 x 130] = lowpass 65-col window | highpass 65-col window).

Both wavelet passes are PE matmuls with the DATA as the stationary operand
and the banded W windows as the moving operand (the implicit weight load is
~free, so per-matmul cost is just the ~65 streamed columns; contiguous 1-D
out/rhs APs only — strided matmul APs run ~3x slower on real hardware):

  pass1 (column DWT, contract h): ps[w, f*512+h'] += X_chunk.T @ W_window.
    X chunks [128 h x 128 w] stationary; per (w-block, h-chunk, filter) one
    65-col window matmul into a 2-bank PSUM tile [128 x 1024] (bank0 = L,
    bank1 = H; one accumulation group per bank).
  pass2 (row DWT, contract w): identical structure with MT chunks
    [128 w x 128 i] stationary and the SAME W windows -> ps[i, g*512+w'].
  drains: [128x1024] PSUM->SBUF bf16 copies alternating Act/DVE (the only
    PSUM-capable engines), ~16.4us/rep combined, fully overlapped with PE.
  DMA (the ~25us/rep bottleneck): loads on the SP HWDGE queue, 4 stores
    per image (3-dim APs, ~1.7x faster than one fused 4-dim AP) on the
    Pool SWDGE queue so store waits never block loads or drains.
  W consts load once, OUTSIDE the benchmark loop body (an in-loop reload's
  WAR hazard stalls the SP queue behind the whole previous iteration).
  The timing build unrolls 32 reps per For_i iteration: cross-iteration
  deps are coarsened at the loop boundary, so a 1-rep body serializes
  loads behind the previous rep's full compute (~38us/rep vs ~25us).
  Output HBM layout [img][chan][h'][w'] bf16; host transposes to NHWC f32.

Per-core traffic is 8.4MB (bf16 in+out) so the wall is the measured mixed
load/store DMA rate (~330GB/s); PE streams ~33k cols (~12.4us) and drains
~16.4us, both hidden under DMA.
"""
import math

import numpy as np
import ml_dtypes

import concourse.bass as bass
import concourse.tile as tile
from concourse import bacc, mybir
from concourse.bass_utils import run_bass_kernel_spmd

N_CORES = 8
IMGS = 2
IMG_ELEMS = 1024 * 1024
OUT_ELEMS = 4 * 512 * 512
F32 = mybir.dt.float32
BF16 = mybir.dt.bfloat16
NPBF16 = ml_dtypes.bfloat16

_S3 = math.sqrt(3.0)
_DEN = 4.0 * math.sqrt(2.0)
H4 = np.array([(1 + _S3) / _DEN, (3 + _S3) / _DEN,
               (3 - _S3) / _DEN, (1 - _S3) / _DEN], dtype=np.float64)
G4 = np.array([H4[3], -H4[2], H4[1], -H4[0]], dtype=np.float64)


def _make_wmat():
    """[128, 260] f32: cols 0-129 = W0 (t=0, mirror baked), 130-259 = Wn."""
    W = np.zeros((128, 130), dtype=np.float64)
    for c in range(65):
        for k in range(4):
            r = 2 * c + k - 2
            if 0 <= r < 128:
                W[r, c] += H4[k]
                W[r, 65 + c] += G4[k]
    W0 = W.copy()
    W0[1, 0] += H4[0]
    W0[0, 0] += H4[1]
    W0[1, 65] += G4[0]
    W0[0, 65] += G4[1]
    return np.concatenate([W0, W], axis=1).astype(np.float32)


def _ap(handle, offset, dims):
    return bass.AP(handle, offset, [list(d) for d in dims])


def _tap(t, off, dims, pcnt=128, poff=0):
    f = t[:]
    pitch = f.ap[0][0]
    return bass.AP(f.tensor, f.offset + poff * pitch + off,
                   [[pitch, pcnt]] + [list(d) for d in dims])


# drain-engine schedule per image (16 drains: 8 pass1 + 8 pass2).
# GPSIMD can't read PSUM, so only Act (~1.04us) and DVE (~1.19us) drain.
_DRAIN_SCHED = ["a", "v", "a", "v", "a", "v", "a", "v",
                "a", "v", "a", "v", "a", "v", "a", "v"]


def _build(reps=1, loop=False, variant="full", unroll=32):
    nc = bacc.Bacc("TRN2", target_bir_lowering=False, debug=False,
                   num_devices=1)
    xh = nc.dram_tensor("x", [IMGS * IMG_ELEMS], BF16, kind="ExternalInput")
    wh = nc.dram_tensor("wmat", [128 * 260], BF16, kind="ExternalInput")
    yh = nc.dram_tensor("y", [IMGS * OUT_ELEMS], BF16, kind="ExternalOutput")

    with tile.TileContext(nc) as tc:
        with (
            tc.tile_pool(name="xs", bufs=4) as px,
            tc.tile_pool(name="mt", bufs=4) as pmt,
            tc.tile_pool(name="yb", bufs=3) as py,
            tc.tile_pool(name="cst", bufs=1) as pc,
            tc.tile_pool(name="pp", bufs=4, space="PSUM") as pp,
        ):
            def drain(eng, dst, src):
                if eng == "a":
                    nc.scalar.activation(
                        dst, src, mybir.ActivationFunctionType.Copy)
                elif eng == "v":
                    nc.vector.tensor_copy(dst, src)
                else:
                    nc.gpsimd.tensor_copy(dst, src)

            def band_group(ps, stat_of, Wt):
                """Accumulate one full DWT line group into psum tile ps.

                ps [128 x 1024] f32 (2 banks), plain layout: bank0 =
                lowpass cols 0..511, bank1 = highpass cols 512..1023.
                Two accumulation groups (one per bank), windows of chunk k
                at 64k..64k+64 stay within their bank — contiguous APs only.
                stat_of(k) -> stationary AP for chunk k.
                """
                for f in range(2):
                    for k in range(8):
                        n = 65 if k < 7 else 64
                        wofs = (0 if k == 0 else 130) + f * 65
                        out = _tap(ps, f * 512 + 64 * k, [[1, n]])
                        rhs = _tap(Wt, wofs, [[1, n]])
                        nc.tensor.matmul(out, stat_of(k), rhs,
                                         start=(k == 0), stop=(k == 7),
                                         skip_group_check=True)

            # const W loads ONCE — reloading per rep creates a WAR hazard
            # that stalls the SP load queue behind the whole previous rep
            Wt = pc.tile([128, 260], BF16, tag="wc")
            nc.sync.dma_start(Wt[:], _ap(wh, 0, [[260, 128], [1, 260]]))

            def body():
                dix = [0]

                def next_eng():
                    e = "a" if dix[0] % 2 == 0 else "v"
                    dix[0] += 1
                    return e

                # load both images up front (SP queue)
                Xs = []
                for img in range(IMGS):
                    X = px.tile([128, 8192], BF16, tag="xt")
                    for half in range(2):
                        nc.sync.dma_start(
                            _tap(X, half * 512, [[1024, 8], [1, 512]]),
                            _ap(xh, img * IMG_ELEMS + half * 512,
                                [[1024, 128], [131072, 8], [1, 512]]))
                    Xs.append(X)

                # ---- pass 1 (both images): column DWT -> MT[w, (f, h')]
                # p1(i1) overlaps the p1(i0)-drain barrier so PE never idles
                MTs = []
                for img in range(IMGS):
                    X = Xs[img]
                    MT = pmt.tile([128, 8192], BF16, tag="mtt")
                    for c in range(8):
                        ps = pp.tile([128, 1024], F32, tag="ps")
                        band_group(
                            ps, lambda t: _tap(X, t * 1024 + c * 128,
                                               [[1, 128]]), Wt)
                        drain(next_eng(),
                              _tap(MT, c * 1024, [[1, 1024]]), ps[:])
                    MTs.append(MT)

                if variant == "dbgmt":
                    for img in range(IMGS):
                        nc.gpsimd.dma_start(
                            _ap(yh, img * OUT_ELEMS, [[8192, 128], [1, 8192]]),
                            MTs[img][:])
                    return

                # ---- pass 2 (both images): row DWT -> Y[i, (g, w')] ----
                for img in range(IMGS):
                    MT = MTs[img]
                    Y = py.tile([128, 8192], BF16, tag="yt")
                    for j in range(8):
                        ps = pp.tile([128, 1024], F32, tag="ps")
                        if variant == "nodep":
                            stat = lambda c: _tap(Wt, 64, [[1, 128]])
                        else:
                            stat = lambda c: _tap(MT, c * 1024 + j * 128,
                                                  [[1, 128]])
                        band_group(ps, stat, Wt)
                        # psY (g-major) -> Y[(2g+f)*2048 + hb*512 + w']
                        f, hb = j // 4, j % 4
                        drain(next_eng(),
                              _tap(Y, f * 2048 + hb * 512, [[4096, 2],
                                                            [1, 512]]),
                              ps[:])

                    # four stores per image (3-dim APs stream ~1.7x faster
                    # than the fused 4-dim AP on hardware)
                    for ch in range(4):
                        nc.gpsimd.dma_start(
                            _ap(yh, img * OUT_ELEMS + ch * 262144,
                                [[512, 128], [65536, 4], [1, 512]]),
                            _tap(Y, ch * 2048, [[512, 4], [1, 512]]))

            if loop and reps > 1:
                q, rem = divmod(reps, unroll)
                if q > 1:
                    with tc.For_i(0, q, 1):
                        for _u in range(unroll):
                            body()
                else:
                    rem = reps
                for _rep in range(rem):
                    body()
            else:
                for _rep in range(reps):
                    body()
    nc.compile()
    return nc


_NC_CACHE = {}


def _get_nc(reps=1, loop=False):
    key = (reps, loop)
    if key not in _NC_CACHE:
        _NC_CACHE[key] = _build(reps, loop)
    return _NC_CACHE[key]


def _const_maps():
    return {"wmat": _make_wmat().astype(NPBF16).ravel()}


def kernel(**inputs):
    x = np.asarray(inputs["x"], dtype=np.float32)
    assert x.shape == (16, 1024, 1024, 1), x.shape
    nc = _get_nc(1)
    xb = x.reshape(N_CORES, IMGS * IMG_ELEMS).astype(NPBF16)
    consts = _const_maps()
    in_maps = [{"x": xb[i], **consts} for i in range(N_CORES)]
    res = run_bass_kernel_spmd(nc, in_maps, core_ids=list(range(N_CORES)))
    # y planar [img][chan][h'][w'] bf16 -> [16, h', w', chan] f32
    full = np.stack([np.asarray(res.results[i]["y"]).reshape(
        IMGS, 4, 512, 512) for i in range(N_CORES)])
    out = full.transpose(0, 1, 3, 4, 2).reshape(16, 512, 512, 4)
    return np.ascontiguousarray(out).astype(np.float32)
